# revision 36
# baseline (speedup 1.0000x reference)
"""Multi-head self-attention (16 heads, hd=64, RoPE, causal) on 8 trn2 cores.

Sharding: DP(batch=2) x TP(head-groups=4). Core c handles batch c//4, heads
[4*(c%4), 4*(c%4)+4). Each core computes a row-parallel partial output
yT_partial [1024, 2048]; host sums the 4 partials per batch and transposes.
No device-device communication.

Device kernel works in transposed layout throughout:
  - xT [e, t] streamed from DRAM
  - qT/kT [d_local, t] with per-head de-interleaved RoPE halves (weight rows
    pre-permuted on host so rot1/rot2 operate on contiguous 32-row blocks)
  - scoresT [kt, qt] per head; K=64 matmuls pair-packed via tile_position
  - probs = exp(scoresT) (no max subtraction; scores are O(1) by construction)
  - attnT [vd, qt] = v_aug.T @ probs with M=65 (65th column of v_aug is ones,
    yielding the softmax denominator row for free)
  - yT [e, qt] = woT.T @ attnT_normalized
"""

import sys

for _p in ("/opt/trn_rl_repo",):
    if _p not in sys.path:
        sys.path.insert(0, _p)

import numpy as np

import concourse.bass as bass
import concourse.mybir as mybir
import concourse.tile as tile
from concourse import bacc
from concourse.bass_utils import run_bass_kernel_spmd

F32 = mybir.dt.float32
F32R = mybir.dt.float32r
BF16 = mybir.dt.bfloat16
AF = mybir.ActivationFunctionType

# matmul groups run in float32r (1 cyc/row at N>=256 vs 4 for fp32).
FP32R_GROUPS = {"qkv", "scores", "attnv", "yt", "bcast"}


def _mm(group, ap):
    return ap.bitcast(F32R) if group in FP32R_GROUPS else ap

B, T, E = 2, 2048, 1024
NH, HD = 16, 64
NHL = 4          # heads per core
DL = NHL * HD    # 256 local head dims
NCORES = 8
NEG = -1e9
ROPE_BASE = 10000.0


# ----------------------------------------------------------------- device IR
def build_module(reps=1):
    nc = bacc.Bacc("TRN2", target_bir_lowering=False, debug=False,
                   num_devices=NCORES)

    xt = nc.dram_tensor("xt", [E, T], F32R, kind="ExternalInput").ap()
    wqt = nc.dram_tensor("wqt", [E, DL], F32R, kind="ExternalInput").ap()
    wkt = nc.dram_tensor("wkt", [E, DL], F32R, kind="ExternalInput").ap()
    wvt = nc.dram_tensor("wvt", [E, DL], F32R, kind="ExternalInput").ap()
    wot = nc.dram_tensor("wot", [DL, E], F32R, kind="ExternalInput").ap()
    cd = nc.dram_tensor("cd", [128, T], F32R, kind="ExternalInput").ap()
    sd = nc.dram_tensor("sd", [128, T], F32R, kind="ExternalInput").ap()
    negid = nc.dram_tensor("negid", [128, 128], BF16, kind="ExternalInput").ap()
    stepd = nc.dram_tensor("stepd", [128, 128], BF16, kind="ExternalInput").ap()
    onesd = nc.dram_tensor("onesd", [128, 64], F32R, kind="ExternalInput").ap()
    yt = nc.dram_tensor("yt", [E, T], F32, kind="ExternalOutput").ap()

    with tile.TileContext(nc) as tc:
        for _ in range(reps):
            _body(tc, xt, wqt, wkt, wvt, wot, cd, sd, negid, stepd, onesd, yt)
    nc.compile()
    return nc


def _chunks(qs_rel):
    """512-bank-aligned column chunks of [qs_rel, 1024)."""
    out = []
    if qs_rel < 512:
        out.append((qs_rel, 512))
        out.append((512, 1024))
    else:
        out.append((qs_rel, 1024))
    return out


def _body(tc, xt, wqt, wkt, wvt, wot, cd, sd, negid, stepd, onesd, yt):
    nc = tc.nc
    from contextlib import ExitStack

    with ExitStack() as outer:
        # all SBUF pools at one level: no pool-close gates between phases
        po = outer.enter_context(tc.tile_pool(name="persist", bufs=1))
        wp = outer.enter_context(tc.tile_pool(name="wp", bufs=1))
        xcp = outer.enter_context(tc.tile_pool(name="xcp", bufs=10))
        swpp = outer.enter_context(tc.tile_pool(name="swp", bufs=2))
        expp = outer.enter_context(tc.tile_pool(name="expp", bufs=4))
        dnp = outer.enter_context(tc.tile_pool(name="dnp", bufs=2))
        yp = outer.enter_context(tc.tile_pool(name="yp", bufs=4))

        # persistent tiles
        qk = {}
        for nm in ("q0", "q1", "k0", "k1"):
            qk[nm] = po.tile([128, T], F32R, tag=nm, name=nm)
        v_sb = po.tile([128, 16 * 260], F32R, tag="v")
        wot_sb = [po.tile([128, E], F32R, tag=f"wot{p}", name=f"wot{p}")
                  for p in range(2)]
        at = [po.tile([128, T], F32R, tag=f"at{p}", name=f"at{p}")
              for p in range(2)]
        negi_sb = po.tile([128, 128], BF16, tag="negi")
        step_sb = po.tile([128, 128], BF16, tag="step")
        ones_sb = po.tile([65, 64], F32R, tag="ones")

        w_sb = {}
        w_srcs = {"wq": wqt, "wk": wkt, "wv": wvt}
        for nm in ("wq", "wk", "wv"):
            w_sb[nm] = wp.tile([128, 2048], F32R, tag=nm, name=nm)

        def _wload(nm):
            nc.sync.dma_start(
                out=w_sb[nm][:].rearrange("p (eo d) -> p eo d", eo=8),
                in_=w_srcs[nm].rearrange("(eo p) d -> p eo d", p=128))

        _wload("wq")
        _wload("wk")
        _wload("wv")
        trig = {}
        for nm in ("c", "s"):
            trig[nm] = wp.tile([128, T], F32R, tag=nm, name="trig_" + nm)

        def _late_loads():
            # issued after the first window's x chunks so the DMA queues
            # prioritize what phase 1 needs first
            nc.sync.dma_start(out=trig["c"][:], in_=cd[:])
            nc.sync.dma_start(out=trig["s"][:], in_=sd[:])
            nc.sync.dma_start(out=negi_sb[:], in_=negid[:])
            nc.sync.dma_start(out=step_sb[:], in_=stepd[:])
            for p in range(2):
                nc.sync.dma_start(out=wot_sb[p][:],
                                  in_=wot[p * 128:(p + 1) * 128, :])
            nc.sync.dma_start(out=ones_sb[64:65, :], in_=onesd[0:1, 0:64])
            v_ones_view = v_sb[:].rearrange("p (tt h x) -> p tt h x",
                                            tt=16, h=4)
            nc.gpsimd.dma_start(
                out=v_ones_view[:, :, :, 64:65],
                in_=onesd[:, 0:64].rearrange("p (tt h) -> p tt h",
                                             tt=16)[:, :, :, None])

        # ---------------- phase 1: projections + rope -----------------------
        with tc.tile_pool(name="pp", bufs=1, space="PSUM") as pp:
            for tcx in range(2):
                for half in range(2):
                    c0 = half * 512
                    tw = tcx * 1024 + c0
                    xc = []
                    for eo in range(8):
                        t_ = xcp.tile([128, 512], F32R, tag="xc", name="xc")
                        eng = nc.gpsimd if eo % 2 == 0 else nc.sync
                        eng.dma_start(
                            out=t_[:],
                            in_=xt[eo * 128:(eo + 1) * 128, tw:tw + 512])
                        xc.append(t_)
                    if tcx == 0 and half == 0:
                        _late_loads()
                    ps = {nm: pp.tile([128, 512], F32, tag="qkps", bufs=4,
                                      name="ps_" + nm)
                          for nm in ("q0", "q1", "k0", "k1")}
                    for eo in range(8):
                        for wnm, dh in (("wq", 0), ("wq", 1),
                                        ("wk", 0), ("wk", 1)):
                            dst = ("q" if wnm == "wq" else "k") + str(dh)
                            nc.tensor.matmul(
                                out=ps[dst][:],
                                lhsT=w_sb[wnm][:, eo * 256 + dh * 128:
                                               eo * 256 + dh * 128 + 128],
                                rhs=xc[eo][:],
                                start=(eo == 0), stop=(eo == 7))
                    for i_, nm in enumerate(("q0", "q1", "k0", "k1")):
                        if i_ % 2 == 0:
                            nc.vector.tensor_copy(
                                qk[nm][:, tw:tw + 512], ps[nm][:])
                        else:
                            nc.scalar.copy(
                                qk[nm][:, tw:tw + 512], ps[nm][:])
                    # V: tt-outer, eo-inner so only 2 psum banks needed
                    for tt_ in range(4):
                        psv = pp.tile([128, 256], F32, tag="vps", bufs=4,
                                      name="psv")
                        for eo in range(8):
                            nc.tensor.matmul(
                                out=psv[:],
                                lhsT=xc[eo][:, tt_ * 128:tt_ * 128 + 128],
                                rhs=w_sb["wv"][:, eo * 256:(eo + 1) * 256],
                                start=(eo == 0), stop=(eo == 7))
                        gt = tw // 128 + tt_
                        dst = v_sb[:, gt * 260:(gt + 1) * 260] \
                            .rearrange("p (h x) -> p h x", h=4)
                        src_ = psv[:].rearrange("p (h x) -> p h x", h=4)
                        if tt_ % 2 == 0:
                            nc.vector.tensor_copy(dst[:, :, 0:64],
                                                  src_[:, :, 0:64])
                        else:
                            nc.scalar.copy(dst[:, :, 0:64],
                                           src_[:, :, 0:64])
                # rope this 1024-window right after its projections
                # (pair-0 tiles first so attention can start early)
                for nm in ("q0", "k0", "q1", "k1"):
                    cs = slice(tcx * 1024, tcx * 1024 + 1024)
                    raw = qk[nm]
                    swp = swpp.tile([128, 1024], F32R, tag="swp")
                    for j in range(4):
                        a, b_ = j * 32, (j ^ 1) * 32
                        nc.gpsimd.dma_start(out=swp[a:a + 32, :],
                                            in_=raw[b_:b_ + 32, cs])
                    nc.vector.tensor_mul(raw[:, cs], raw[:, cs],
                                         trig["c"][:, cs])
                    nc.vector.tensor_mul(swp[:], swp[:], trig["s"][:, cs])
                    nc.vector.tensor_add(raw[:, cs], raw[:, cs], swp[:])

        # ---------------- phase 2 + 3: attention, yT interleaved -----------
        with tc.tile_pool(name="ps_s", bufs=2, space="PSUM") as ps_sp, \
             tc.tile_pool(name="ps_a", bufs=2, space="PSUM") as ps_ap:
            recip_pool = dnp
            ytv = yt.rearrange("(et p) t -> p et t", p=128)

            def yt_chunk(qc, pool, tag):
                # output projection for qt cols [qc*512, qc*512+512)
                for eg in range(2):
                    y_sb = yp.tile([128, 4 * 512], F32, tag="ysb", bufs=2,
                                   name="y_sb")
                    for ei in range(4):
                        et_ = eg * 4 + ei
                        ps_y = pool.tile([128, 512], F32, tag=tag,
                                         name="ps_y")
                        for p in range(2):
                            nc.tensor.matmul(
                                out=ps_y[:],
                                lhsT=wot_sb[p][:, et_ * 128:(et_ + 1) * 128],
                                rhs=at[p][:, qc * 512:(qc + 1) * 512],
                                start=(p == 0), stop=(p == 1))
                        if et_ % 2 == 0:
                            nc.vector.tensor_copy(
                                y_sb[:, ei * 512:(ei + 1) * 512], ps_y[:])
                        else:
                            nc.scalar.copy(
                                y_sb[:, ei * 512:(ei + 1) * 512], ps_y[:])
                    eng = nc.gpsimd if qc % 2 == 0 else nc.sync
                    eng.dma_start(
                        out=ytv[:, eg * 4:eg * 4 + 4,
                                qc * 512:(qc + 1) * 512],
                        in_=y_sb[:].rearrange("p (et t) -> p et t", et=4))

            def attention(pair, qtb):
                krot = [qk["k" + str(pair)], qk["k" + str(pair)]]
                qrot = [qk["q" + str(pair)], qk["q" + str(pair)]]
                nkt = 8 * qtb + 8
                q0 = qtb * 1024
                ps_a = [ps_ap.tile([128, 1024], F32, tag="a", name="ps_a")
                        for _ in range(2)]
                exp_tiles = [None] * nkt
                chunk_l = [None] * nkt

                def scores_step(kt):
                    qs_rel = max(0, 128 * kt - q0)
                    chs = _chunks(qs_rel)
                    chunk_l[kt] = chs
                    ps_s = [ps_sp.tile([128, 1024], F32, tag="s",
                                       name="ps_s")
                            for _ in range(2)]
                    diag = kt >= 8 * qtb
                    for h in range(2):
                        for (ca, cb) in chs:
                            is_diag_chunk = diag and ca == qs_rel
                            nc.tensor.matmul(
                                out=ps_s[h][:, ca:cb],
                                lhsT=krot[h][h * 64:h * 64 + 64,
                                             kt * 128:kt * 128 + 128],
                                rhs=qrot[h][h * 64:h * 64 + 64,
                                            q0 + ca:q0 + cb],
                                start=True, stop=not is_diag_chunk,
                                tile_position=(h * 64, 0))
                    if diag:
                        for h in range(2):
                            nc.tensor.matmul(
                                out=ps_s[h][:, qs_rel:qs_rel + 128],
                                lhsT=negi_sb[:],
                                rhs=step_sb[:],
                                start=False, stop=True)
                    et = []
                    for h in range(2):
                        e_ = expp.tile([128, 1024], F32R, tag="e",
                                       name="exp_t")
                        nc.scalar.activation(
                            e_[:, qs_rel:1024], ps_s[h][:, qs_rel:1024],
                            AF.Exp)
                        et.append(e_)
                    exp_tiles[kt] = et

                def attnv_step(kt):
                    # psum stop flags are bank-granular: bank0's last
                    # writer is kt=8*qtb+3, bank1's is nkt-1
                    et = exp_tiles[kt]
                    for h in range(2):
                        slot = kt * 260 + (2 * pair + h) * 65
                        for (ca, cb) in chunk_l[kt]:
                            last = 8 * qtb + 3 if ca < 512 else nkt - 1
                            nc.tensor.matmul(
                                out=ps_a[h][0:65, ca:cb],
                                lhsT=v_sb[:, slot:slot + 65],
                                rhs=et[h][:, ca:cb],
                                start=(kt == 0), stop=(kt == last))
                    exp_tiles[kt] = None

                for step in range(nkt + 1):
                    if step < nkt:
                        scores_step(step)
                    if step > 0:
                        attnv_step(step - 1)

                # normalization: denom rows staged in f32r dh tiles
                # (they feed the K=1 broadcast matmul); recip outputs in
                # plain f32 tiles (only consumed by DVE)
                dh0 = recip_pool.tile([128, 1024], F32R, tag="dh",
                                      name="dh0")
                dh1 = recip_pool.tile([128, 1024], F32R, tag="dh",
                                      name="dh1")
                nc.scalar.copy(dh0[64:65, :], ps_a[0][64:65, :])
                nc.scalar.copy(dh1[64:65, :], ps_a[1][64:65, :])
                ps_b = [ps_sp.tile([128, 1024], F32, tag="s",
                                   name="ps_b") for _ in range(2)]
                dns = (dh0, dh1)
                for h in range(2):
                    for (ca, cb) in ((0, 512), (512, 1024)):
                        nc.tensor.matmul(
                            out=ps_b[h][0:64, ca:cb],
                            lhsT=ones_sb[64:65, :],
                            rhs=dns[h][64:65, ca:cb],
                            start=True, stop=True,
                            tile_position=(64, 0))
                recip = recip_pool.tile([128, 1024], F32, tag="rc")
                scr = recip_pool.tile([128, 1024], F32, tag="scr")
                nc.vector.reciprocal_approx_fast(
                    out=recip[0:64, :], in_=ps_b[0][0:64, :])
                nc.vector.reciprocal_approx_fast(
                    out=scr[0:64, :], in_=ps_b[1][0:64, :])
                nc.vector.tensor_mul(
                    at[pair][0:64, q0:q0 + 1024],
                    ps_a[0][0:64, :], recip[0:64, :])
                a1n = recip_pool.tile([64, 1024], F32R, tag="dh",
                                      name="a1n")
                nc.vector.tensor_mul(
                    a1n[0:64, :], ps_a[1][0:64, :], scr[0:64, :])
                nc.gpsimd.dma_start(
                    out=at[pair][64:128, q0:q0 + 1024], in_=a1n[0:64, :])

            for pair in range(2):
                for qtb in range(2):
                    attention(pair, qtb)

        # ---------------- phase 3: output projection ------------------------
        with tc.tile_pool(name="ps_y", bufs=4, space="PSUM") as ps_yp:
            for qc in range(4):
                yt_chunk(qc, ps_yp, "y")


# ----------------------------------------------------------------- host side
def _prep_core_inputs(x, wq, wk, wv, wo):
    """Build the 8 per-core input dicts (numpy fp32)."""
    # rope trig tables, transposed [freq, pos]
    inv_freq = 1.0 / (ROPE_BASE ** (np.arange(0, HD, 2, dtype=np.float32) / HD))
    pos = np.arange(T, dtype=np.float32)
    freqs = pos[:, None] * inv_freq[None, :]          # [T, 32]
    cosT = np.cos(freqs).T.astype(np.float32)          # [32, T]
    sinT = np.sin(freqs).T.astype(np.float32)
    C = np.tile(cosT, (4, 1)).astype(np.float32)       # [128, T]
    S = np.tile(np.concatenate([-sinT, sinT], axis=0), (2, 1)).astype(np.float32)
    scale = np.float32(1.0 / np.sqrt(HD))              # folded into wq

    import ml_dtypes
    r, c = np.indices((128, 128))
    negid = (np.eye(128) * NEG).astype(ml_dtypes.bfloat16)
    stepd = (c < r).astype(ml_dtypes.bfloat16)

    # per-head de-interleave: rows [even dims, odd dims]
    perm = np.concatenate([np.arange(0, HD, 2), np.arange(1, HD, 2)])

    in_maps = []
    for core in range(NCORES):
        b_, hg = divmod(core, 4)
        heads = np.arange(4 * hg, 4 * hg + 4)
        rows = np.concatenate([h * HD + perm for h in heads])      # permuted
        rows_plain = np.concatenate([h * HD + np.arange(HD) for h in heads])
        xt = np.ascontiguousarray(x[b_].T)                         # [E, T]
        wqt_ = np.ascontiguousarray(wq[rows, :].T) * scale         # [E, DL]
        wkt_ = np.ascontiguousarray(wk[rows, :].T)
        wvt_ = np.ascontiguousarray(wv[rows_plain, :].T)
        wot_ = np.ascontiguousarray(wo[:, rows_plain].T)           # [DL, E]
        in_maps.append({
            "xt": xt, "wqt": wqt_, "wkt": wkt_, "wvt": wvt_, "wot": wot_,
            "cd": C, "sd": S,
            "negid": negid, "stepd": stepd,
            "onesd": np.ones((128, 64), dtype=np.float32),
        })
    return in_maps


_NC_CACHE = {}


def _get_module():
    if "nc" not in _NC_CACHE:
        _NC_CACHE["nc"] = build_module()
    return _NC_CACHE["nc"]


def _get_runner(key="nc", builder=None):
    """Build (once) a cached jax.jit shard_map callable over the 8 cores."""
    rkey = "runner_" + key
    if rkey in _NC_CACHE:
        return _NC_CACHE[rkey]
    import jax
    import concourse.mybir as _mb
    from concourse import bass2jax as b2j
    from jax.sharding import Mesh, PartitionSpec
    from jax.experimental.shard_map import shard_map

    if key == "nc":
        nc = _get_module()
    else:
        if key not in _NC_CACHE:
            _NC_CACHE[key] = builder()
        nc = _NC_CACHE[key]
    b2j.install_neuronx_cc_hook()
    partition_name = (nc.partition_id_tensor.name
                      if nc.partition_id_tensor else None)
    in_names, out_names, out_avals, zero_outs = [], [], [], []
    for alloc in nc.m.functions[0].allocations:
        if not isinstance(alloc, _mb.MemoryLocationSet):
            continue
        name = alloc.memorylocations[0].name
        if alloc.kind == "ExternalInput":
            if name != partition_name:
                in_names.append(name)
        elif alloc.kind == "ExternalOutput":
            out_names.append(name)
            shape = tuple(alloc.tensor_shape)
            dtype = _mb.dt.np(alloc.dtype)
            out_avals.append(jax.core.ShapedArray(shape, dtype))
            zero_outs.append(np.zeros(shape, dtype))
    n_params = len(in_names)
    all_names = list(in_names) + list(out_names)
    if partition_name is not None:
        all_names.append(partition_name)

    def _body(*args):
        operands = list(args)
        if partition_name is not None:
            operands.append(b2j.partition_id_tensor())
        outs = b2j._bass_exec_p.bind(
            *operands,
            out_avals=tuple(out_avals),
            in_names=tuple(all_names),
            out_names=tuple(out_names),
            lowering_input_output_aliases=(),
            sim_require_finite=True,
            sim_require_nnan=True,
            nc=nc,
        )
        return tuple(outs)

    devices = jax.devices()[:NCORES]
    mesh = Mesh(np.asarray(devices), ("core",))
    n_outs = len(out_names)
    in_specs = (PartitionSpec("core"),) * (n_params + n_outs)
    out_specs = (PartitionSpec("core"),) * n_outs
    sharded = jax.jit(
        shard_map(_body, mesh=mesh, in_specs=in_specs, out_specs=out_specs,
                  check_rep=False),
        keep_unused=True)
    from jax.sharding import NamedSharding
    _shard = NamedSharding(mesh, PartitionSpec("core"))
    concat_zeros = [
        jax.device_put(
            np.zeros((NCORES * z.shape[0], *z.shape[1:]), z.dtype), _shard)
        for z in zero_outs
    ]
    runner = {
        "sharded": sharded, "in_names": in_names, "out_names": out_names,
        "out_avals": out_avals, "concat_zeros": concat_zeros,
    }
    _NC_CACHE[rkey] = runner
    return runner


def _run_spmd_cached(in_maps):
    r = _get_runner()
    concat_in = [
        np.concatenate([np.asarray(in_maps[c][nm]) for c in range(NCORES)],
                       axis=0)
        for nm in r["in_names"]
    ]
    out_arrs = r["sharded"](*concat_in, *r["concat_zeros"])
    nm = r["out_names"]
    av = r["out_avals"]
    return [
        {nm[i]: np.asarray(out_arrs[i]).reshape(NCORES, *av[i].shape)[c]
         for i in range(len(nm))}
        for c in range(NCORES)
    ]


def _build_trivial():
    nc = bacc.Bacc("TRN2", target_bir_lowering=False, debug=False,
                   num_devices=NCORES)
    a = nc.dram_tensor("a", [128, 128], F32, kind="ExternalInput").ap()
    b_ = nc.dram_tensor("b", [128, 128], F32, kind="ExternalOutput").ap()
    with tile.TileContext(nc) as tc:
        with tc.tile_pool(name="t", bufs=1) as p:
            t_ = p.tile([128, 128], F32, tag="t")
            nc.sync.dma_start(out=t_[:], in_=a[:])
            nc.sync.dma_start(out=b_[:], in_=t_[:])
    nc.compile()
    return nc


def bench_hw(x, wq, wk, wv, wo, reps=9, n=30):
    """HW per-iteration time from slope: module with body repeated `reps`
    times vs once, both on the same dispatch floor."""
    import time
    import jax
    from jax.sharding import Mesh, NamedSharding, PartitionSpec

    mesh = Mesh(np.asarray(jax.devices()[:NCORES]), ("core",))
    shard = NamedSharding(mesh, PartitionSpec("core"))

    def timed(runner, concat_in):
        f = runner["sharded"]
        zs = runner["concat_zeros"]
        out = f(*concat_in, *zs)
        out[0].block_until_ready()
        ts = []
        for _ in range(n):
            t0 = time.perf_counter()
            o = f(*concat_in, *zs)
            o[0].block_until_ready()
            ts.append(time.perf_counter() - t0)
        ts.sort()
        return ts[0], ts[len(ts) // 2]

    in_maps = _prep_core_inputs(x, wq, wk, wv, wo)

    def concat(runner):
        return [
            jax.device_put(np.concatenate(
                [np.asarray(in_maps[c][nm]) for c in range(NCORES)], axis=0),
                shard)
            for nm in runner["in_names"]
        ]

    r1 = _get_runner()
    t1_min, t1_med = timed(r1, concat(r1))
    rR = _get_runner(f"nc_r{reps}", lambda: build_module(reps=reps))
    tR_min, tR_med = timed(rR, concat(rR))
    per_min = (tR_min - t1_min) / (reps - 1)
    per_med = (tR_med - t1_med) / (reps - 1)
    print(f"  x1: min {t1_min*1e3:.3f} med {t1_med*1e3:.3f} ms ; "
          f"x{reps}: min {tR_min*1e3:.3f} med {tR_med*1e3:.3f} ms")
    print(f"HW exec time: {per_min*1e9:.0f} ns (min)  {per_med*1e9:.0f} ns (med)")
    return per_min


def kernel(x, wq, wk, wv, wo, _trace=False, _trace_kwargs=None):
    x = np.asarray(x, dtype=np.float32)
    wq = np.asarray(wq, dtype=np.float32)
    wk = np.asarray(wk, dtype=np.float32)
    wv = np.asarray(wv, dtype=np.float32)
    wo = np.asarray(wo, dtype=np.float32)

    in_maps = _prep_core_inputs(x, wq, wk, wv, wo)
    try:
        results = _run_spmd_cached(in_maps)
    except Exception:
        nc = _get_module()
        results = run_bass_kernel_spmd(
            nc, in_maps, core_ids=list(range(NCORES))).results
    out = np.empty((B, T, E), dtype=np.float32)
    for b_ in range(B):
        acc = np.zeros((E, T), dtype=np.float32)
        for g in range(4):
            acc += results[4 * b_ + g]["yt"]
        out[b_] = acc.T
    return out


if __name__ == "__main__":
    nc = _get_module()
    print("module built ok")
